# revision 3
# baseline (speedup 1.0000x reference)
"""ColBERT max-sim kernel v2: 3-lane PSUM drain (DVE + ACT/DVE-folds + Pool).

Math: out[q,d] = sum_l max_t sum_e doc[d,t,e] * query[q,e,l]

Docs sharded 16/core, queries replicated. Per core, the 65536 "columns"
(128 t-scores each, one per (q,l,d)) are drained from PSUM by three lanes:
  lane a (docs 0-6):   DVE tensor_reduce(max) straight from PSUM.
  lane b (docs 7-9):   ACT copy PSUM->bf16 SBUF, lagged DVE fold tree (2x_1p).
  lane c (docs 10-15): TRANSPOSED matmuls (psum [128 t, ql-chunk]), ACT copy
      -> bf16 SBUF doc tile cT [128, 4096], Pool (GpSimd) axis=C partition-max
      -> [1, 4096] row, DMA-scattered into the maxq quarter tiles.
Sum over l for all 16 docs: per 8-group quarter, PE matmul with block-diag
ones lsum [128, 4] (contraction over the 128 (m,l) partitions), DVE moves
psum->SBUF, DMA out.  The lsum psum tiles ride the pd pool rotation.

PSUM budget (16 KiB/partition, 2 KiB banks): pd [128,10,128] f32 x2 bufs
(10 KiB) + pcA [128,2,512] (4 KiB) + pcB [128,512] (2 KiB) = 16 KiB.
Lane-c chunk pattern per doc: A(1024) B(512) A(1024) B(512) A(1024),
one chunk per group; ACT copies lag one group; Pool fires at group 5r+6.
"""

import numpy as np
import ml_dtypes

import concourse.bass as bass
import concourse.tile as tile
from concourse import mybir
from concourse.bass_utils import run_bass_kernel_spmd
from concourse.vector_clock import ScopedClock

N_CORES = 8
ND, LD, E = 128, 128, 128      # docs, doc tokens, embed dim
NQ, LQ = 128, 32               # queries, query tokens
NDC = ND // N_CORES            # docs per core = 16
NG = 32                        # M-groups (4 queries x 32 l each)
QG = 4                         # queries per group
F32 = mybir.dt.float32
BF16 = mybir.dt.bfloat16
MAXOP = mybir.AluOpType.max
ADDOP = mybir.AluOpType.add
AXX = mybir.AxisListType.X
AXC = mybir.AxisListType.C

NA = 8                         # lane-a docs (DVE direct reduce)
NB = 2                         # lane-b docs (ACT copy + DVE folds)
NAB = NA + NB                  # normal-lane docs per group = 10
NCD = NDC - NAB                # lane-c docs (transposed + Pool) = 6

# lane-c: eight 512-col chunks per doc on a double-buffered 1-bank psum
# pool; 48 chunks front-loaded to all be issued by group C_LASTG.
C_COLS = 512
C_PER_DOC = NQ * LQ // C_COLS          # 8
C_TOTAL = NCD * C_PER_DOC              # 56
N_DVE_COPIES = 2               # first chunks copied by DVE during its
                               # DMA-wait window at the start
C_LASTG = 21

_MAX_DRAIN_WAITS = 1


def _patched_drain_and_barrier(self, tick_clock, wait_clock):
    nc = self.nc
    drain_inst = nc.sync.drain()
    wait_clock.add_sem_waits(
        drain_inst.ins, ScopedClock({None: tick_clock.global_clock})
    )
    si = drain_inst.ins.sync_info
    waits = list(si.on_wait) if si is not None and si.on_wait else []
    if len(waits) > _MAX_DRAIN_WAITS:
        si.on_wait = waits[:_MAX_DRAIN_WAITS]
        drain_inst.ins.sync_info = si
        rest = waits[_MAX_DRAIN_WAITS:]
        while rest:
            extra = nc.sync.drain()
            esi = extra.ins.sync_info
            if esi is None:
                esi = si
            esi.on_wait = rest[:_MAX_DRAIN_WAITS]
            esi.on_update = []
            extra.ins.sync_info = esi
            rest = rest[_MAX_DRAIN_WAITS:]
    nc.all_engine_barrier()
    assert self.sems is not None
    popped = nc._tile_sem_poison_stack.pop()
    assert popped is self._sem_poison
    nc.clear_and_free_semaphores(list(self.sems.allocated().values()))
    nc.all_engine_barrier()


def _apply_tile_patch():
    if getattr(tile.TileContext, "_drain_patch_applied", False):
        return
    tile.TileContext._drain_and_barrier = _patched_drain_and_barrier
    tile.TileContext._drain_patch_applied = True


def _split_excess_waits(nc, max_waits=_MAX_DRAIN_WAITS):
    """walrus rejects instructions with too many sem waits; move the excess
    onto NoOp carriers inserted immediately before on the same engine."""
    for f in nc.m.functions:
        for blk in f.blocks:
            snapshot = list(blk.instructions)
            for idx in range(len(snapshot) - 1, -1, -1):
                inst = snapshot[idx]
                limit = max_waits
                si = getattr(inst, "sync_info", None)
                if si is None or not si.on_wait or len(si.on_wait) <= limit:
                    continue
                waits = list(si.on_wait)
                si.on_wait = waits[-limit:]
                inst.sync_info = si
                rest = waits[:-limit]
                chunks = [
                    rest[i : i + max_waits] for i in range(0, len(rest), max_waits)
                ]
                for chunk in reversed(chunks):
                    noop = mybir.InstNoOp(
                        name=nc.get_next_instruction_name(),
                        engine=inst.engine,
                        bass_nofuse=True,
                    )
                    noop.sync_info = mybir.SyncInfo(on_wait=chunk, on_update=[])
                    nc.register_instruction(noop)
                    blk.instructions.insert(idx, noop)


def _build_nc():
    _apply_tile_patch()
    nc = bass.Bass("TRN2", target_bir_lowering=False, debug=False)
    d_dram = nc.dram_tensor("docT", [E, NDC, LD], BF16, kind="ExternalInput")
    q_dram = nc.dram_tensor("qT", [E, NQ * LQ], BF16, kind="ExternalInput")
    l_dram = nc.dram_tensor("lsum", [128, QG], BF16, kind="ExternalInput")
    scratch = nc.dram_tensor("cscratch", [2 * NCD, NQ * LQ // 2], BF16, kind="Internal")
    out_dram = nc.dram_tensor("out", [QG, NG, NDC], F32, kind="ExternalOutput")

    with tile.TileContext(nc) as tc:
        with (
            tc.tile_pool(name="const", bufs=1) as const_pool,
            tc.tile_pool(name="cbuf", bufs=2) as cbuf_pool,
            tc.tile_pool(name="fold", bufs=4) as fold_pool,
            tc.tile_pool(name="ct", bufs=3) as ct_pool,
            tc.tile_pool(name="trow", bufs=2) as trow_pool,
            tc.tile_pool(name="osb", bufs=2) as osb_pool,
            tc.tile_pool(name="pd", bufs=2, space="PSUM") as pd_pool,
            tc.tile_pool(name="pdb", bufs=2, space="PSUM") as pdb_pool,
            tc.tile_pool(name="pca", bufs=2, space="PSUM") as pca_pool,
        ):
            qsb = const_pool.tile([E, NQ * LQ], BF16)
            dsb = const_pool.tile([E, NDC, LD], BF16)
            lsum = const_pool.tile([128, QG], BF16)
            maxq = [
                const_pool.tile([128, 8, NDC], BF16, name=f"maxq{c}")
                for c in range(4)
            ]

            def _maxq_write(g0, n, dlo, dhi):
                # view over group range [g0, g0+n) x docs [dlo, dhi): only
                # valid within one quarter tile
                c = g0 // 8
                assert (g0 + n - 1) // 8 == c
                return maxq[c][:, g0 % 8 : g0 % 8 + n, dlo:dhi]
            warm = const_pool.tile([E, 512], BF16)

            # DMA issuance spread across rings for queue parallelism.
            # SP issuance is engine-free; one chunk each on the DVE and Pool
            # rings (those engines are idle at start; ring issuance blocks
            # the engine ~1us).  qT[0:1024] first: it gates both group 0 and
            # the first lane-c chunk.
            nc.sync.dma_start(dsb[:, 0:8, :], d_dram[:, 0:8, :])
            nc.scalar.dma_start(qsb[:, 0:512], q_dram[:, 0:512])
            nc.gpsimd.dma_start(qsb[:, 512:1536], q_dram[:, 512:1536])
            nc.sync.dma_start(dsb[:, 8:16, :], d_dram[:, 8:16, :])
            nc.sync.dma_start(qsb[:, 1536:2560], q_dram[:, 1536:2560])
            nc.sync.dma_start(qsb[:, 2560:3584], q_dram[:, 2560:3584])
            nc.sync.dma_start(qsb[:, 3584:4096], q_dram[:, 3584:4096])
            nc.sync.dma_start(lsum[:], l_dram[:])

            # PE warmup: ramp the clock p-state while DMAs are in flight.
            nc.vector.memset(warm[:], 0.0)
            for w in range(2):
                wps = pd_pool.tile([128, NA, LD], F32, tag="pd")
                for k in range(2):
                    nc.tensor.matmul(
                        wps[:, 4 * k : 4 * k + 4, :],
                        warm[:, 0:128],
                        warm[:].rearrange("e (d t) -> e d t", d=4),
                    )

            # ---- lane-c chunk schedule: 48 chunks, front-loaded ----
            # chunk k: doc r = k // C_PER_DOC, half h = (k % C_PER_DOC) // 4,
            # col offset within half = 512 * (k % 4).
            ct_half = {}           # (r, h) -> cth tile [128, 2048] bf16
            pc_live = {}           # k -> psum tile
            done_halves = {}

            def _chunk_rh(k):
                # half-major order: chunks 0..23 cover (r, h=0), 24..47 (r, h=1)
                h = k // (4 * NCD)
                r = (k % (4 * NCD)) // 4
                j = k % 4
                return r, h, j

            def _emit_c_matmul(k):
                r, h, j = _chunk_rh(k)
                d = NAB + r
                off = 2048 * h + C_COLS * j
                pc = pca_pool.tile([128, C_COLS], F32, tag="pcA")
                nc.tensor.matmul(pc[:], dsb[:, d, :], qsb[:, off : off + C_COLS])
                pc_live[k] = pc

            def _emit_c_copy(k):
                r, h, j = _chunk_rh(k)
                if (r, h) not in ct_half:
                    ct_half[(r, h)] = ct_pool.tile(
                        [128, NQ * LQ // 2], BF16, tag="ct", name=f"cT{r}_{h}")
                pc = pc_live.pop(k)
                dst = ct_half[(r, h)][:, C_COLS * j : C_COLS * (j + 1)]
                if k < N_DVE_COPIES:
                    # DVE is idle while the input DMAs land: put the first
                    # chunk copies there to warm up the Pool feed early
                    nc.vector.tensor_scalar_add(dst, pc[:], 0.0)
                else:
                    nc.scalar.copy(dst, pc[:])
                if j == 3:
                    _emit_c_pool(r, h)

            def _emit_c_pool(r, h):
                # Pool: partition-max over t for one half-doc -> [1, 2048],
                # bounce via DRAM, scatter into maxq groups 16h..16h+16.
                # The last doc goes in two quarter-row pieces so the Pool
                # drain and the DMA chain start sooner, on the idle ACT ring.
                ct = ct_half.pop((r, h))
                d = NAB + r
                i = 2 * r + h
                last = r == NCD - 1 and h == 1
                pieces = 2 if last else 1
                w = NQ * LQ // 2 // pieces
                for piece in range(pieces):
                    row = trow_pool.tile([1, w], BF16, tag="trow")
                    nc.gpsimd.tensor_reduce(
                        row[:], ct[:, piece * w : (piece + 1) * w],
                        op=MAXOP, axis=AXC)
                    nc.sync.dma_start(
                        scratch[i : i + 1, piece * w : (piece + 1) * w],
                        row[:])
                    for cc in (range(piece, piece + 1) if pieces == 2
                               else range(2)):
                        c = 2 * h + cc
                        nc.sync.dma_start(
                            maxq[c][:, :, d],
                            scratch[i, 1024 * cc : 1024 * (cc + 1)].rearrange(
                                "(j p) -> p j", j=8),
                        )
                done_halves[h] = done_halves.get(h, 0) + 1

            # ---- lsum output per quarter ----
            out_view = out_dram[:].rearrange("m g d -> m (g d)")
            lsum_done = set()

            outsb_all = [None]

            def _emit_lsum(c):
                if c in lsum_done:
                    return
                lsum_done.add(c)
                if outsb_all[0] is None:
                    outsb_all[0] = osb_pool.tile(
                        [QG, 4, 8 * NDC], F32, tag="ob", name="outsb")
                pt = pca_pool.tile([QG, 8 * NDC], F32, tag="pcA")
                nc.tensor.matmul(pt[:], lsum[:], maxq[c][:])
                nc.scalar.copy(outsb_all[0][:, c, :], pt[:])
                if len(lsum_done) == 4:
                    nc.sync.dma_start(
                        out_view[:].rearrange("m (c x) -> m c x", c=4),
                        outsb_all[0][:])

            # ---- lane-b folds: octet-batched (8 groups per chain) ----
            cb_tiles = {}          # octet o -> cb tile [128, 8, NB, 128] bf16

            def _emit_octet(o):
                cb = cb_tiles.pop(o)
                n = cb.shape[1]
                f1 = fold_pool.tile([128, n, NB, 64], BF16, tag="f1")
                nc.vector.tensor_tensor(
                    f1[:], cb[:, :, :, 0:64], cb[:, :, :, 64:128], op=MAXOP)
                f2 = fold_pool.tile([128, n, NB, 32], BF16, tag="f2")
                nc.vector.tensor_tensor(
                    f2[:], f1[:, :, :, 0:32], f1[:, :, :, 32:64], op=MAXOP)
                f3 = fold_pool.tile([128, n, NB, 16], BF16, tag="f3")
                nc.vector.tensor_tensor(
                    f3[:], f2[:, :, :, 0:16], f2[:, :, :, 16:32], op=MAXOP)
                f4 = fold_pool.tile([128, n, NB, 8], BF16, tag="f4")
                nc.vector.tensor_tensor(
                    f4[:], f3[:, :, :, 0:8], f3[:, :, :, 8:16], op=MAXOP)
                nc.vector.tensor_reduce(
                    maxq[o][:, 0:n, NA:NAB],
                    f4[:], op=MAXOP, axis=AXX,
                )

            # ---- group loop ----
            pending = []
            issued = [0]

            def _tick(g):
                # lane-c: issue this group's chunk matmuls, copying the
                # oldest pending chunk before each (bufs=2 keeps PE fed).
                want = min(C_TOTAL, (g + 1) * C_TOTAL // C_LASTG) if g < NG else C_TOTAL
                while issued[0] < want:
                    if pending:
                        _emit_c_copy(pending.pop(0))
                    _emit_c_matmul(issued[0])
                    pending.append(issued[0])
                    issued[0] += 1
                # octet folds: octet o at group 8o+8; the quad over
                # groups 24-27 fires at 29
                if g >= 8 and g % 8 == 0 and (g - 8) // 8 < 3:
                    _emit_octet((g - 8) // 8)
                if g == 29:
                    _emit_octet(3)

            NBG = 28               # groups with a lane-b share
            for g in range(NG):
                lhsT = qsb[:, 128 * g : 128 * (g + 1)]
                pda = pd_pool.tile([128, NA, LD], F32, tag="pd")
                nc.tensor.matmul(pda[:, 0:4, :], lhsT, dsb[:, 0:4, :])
                nc.tensor.matmul(pda[:, 4:8, :], lhsT, dsb[:, 4:8, :])
                if g % 2 == 0:
                    pdb_pair = pdb_pool.tile(
                        [128, 2, NB, LD], F32, tag="pdb")
                nc.tensor.matmul(
                    pdb_pair[:, g % 2, :, :], lhsT, dsb[:, NA:NAB, :],
                    start=(g % 2 == 0), stop=(g % 2 == 1),
                    skip_group_check=True,
                )
                pdb = pdb_pair[:, g % 2, :, :]
                _tick(g)
                # lane a: DVE direct segmented reduce (only needs pda)
                nc.vector.tensor_reduce(
                    _maxq_write(g, 1, 0, NA).rearrange("p o d -> p (o d)"),
                    pda[:], op=MAXOP, axis=AXX,
                )
                if g < NBG:
                    # lane b: one ACT copy per pdb pair into the cb tile
                    o = g // 8
                    if g % 8 == 0:
                        nslots = min(8, NBG - 8 * o)
                        cb_tiles[o] = cbuf_pool.tile(
                            [128, nslots, NB, LD], BF16, tag="cb",
                            name=f"cb{o}")
                    if g % 2 == 1:
                        nc.scalar.copy(
                            cb_tiles[o][:, g % 8 - 1 : g % 8 + 1, :, :],
                            pdb_pair[:],
                        )
                else:
                    # tail groups: DVE reduces docs NA..NAB directly per
                    # pdb pair (no lane-b chain to drain at the end)
                    if g % 2 == 1:
                        nc.vector.tensor_reduce(
                            _maxq_write(g - 1, 2, NA, NAB),
                            pdb_pair[:], op=MAXOP, axis=AXX,
                        )

            for g in range(NG, NG + 10):
                _tick(g)
                if pending:
                    _emit_c_copy(pending.pop(0))

            # remaining lsum quarters (0-2 were emitted mid-loop as their
            # inputs completed)
            for c in range(4):
                _emit_lsum(c)

    _split_excess_waits(nc)
    return nc


_NC_CACHE = None


def _get_nc():
    global _NC_CACHE
    if _NC_CACHE is None:
        _NC_CACHE = _build_nc()
    return _NC_CACHE


def _prep(doc_tokens, query_tokens):
    doc = np.ascontiguousarray(np.asarray(doc_tokens, dtype=np.float32))
    q = np.ascontiguousarray(np.asarray(query_tokens, dtype=np.float32))
    assert doc.shape == (ND, LD, E), doc.shape
    assert q.shape == (NQ, E, LQ), q.shape
    qT = np.ascontiguousarray(
        q.transpose(1, 0, 2).reshape(E, NQ * LQ).astype(ml_dtypes.bfloat16)
    )
    docT = np.ascontiguousarray(
        doc.transpose(2, 0, 1).astype(ml_dtypes.bfloat16)
    )
    lsum = np.zeros((128, QG), dtype=ml_dtypes.bfloat16)
    for m in range(QG):
        lsum[32 * m : 32 * (m + 1), m] = 1.0
    return docT, qT, lsum


def kernel(doc_tokens, query_tokens):
    docT, qT, lsum = _prep(doc_tokens, query_tokens)
    nc = _get_nc()
    in_maps = [
        {
            "docT": np.ascontiguousarray(docT[:, NDC * c : NDC * (c + 1), :]),
            "qT": qT,
            "lsum": lsum,
        }
        for c in range(N_CORES)
    ]
    res = run_bass_kernel_spmd(nc, in_maps, list(range(N_CORES))).results
    cols = []
    for c in range(N_CORES):
        o = res[c]["out"]  # [4, 32, 16]: out[m, g, d] = out_q(4g+m, d)
        cols.append(o.transpose(1, 0, 2).reshape(NQ, NDC))
    return np.ascontiguousarray(np.concatenate(cols, axis=1), dtype=np.float32)
